# revision 3
# baseline (speedup 1.0000x reference)
"""Trainium2 Bass kernel for nn_MultiDense (moe_routing) — v7.

Reference computation:
    p = params[inds_ne]            # [I, 128, 129]
    w = p[..., :128]; b = p[..., 128]
    out[i] = x_in[i] @ w[i].T + b[i]     # [I, 32, 128]

v6 strategy (8 cores):
  - Host routes indices to cores by node range (sorted by node id), groups
    up to G=2 same-node indices into one "item" (64 samples sharing one
    weight matrix). ~640 items/core, capacity 768 (6 chunks of 128).
  - Weights fetched with a NON-transposed dma_gather: each item's whole
    [128x128] block is one contiguous 32KB read landing in one partition
    row (big descriptors; line-rate, unlike the 256B-granular transposed
    gather which is Q7 descriptor-bound).
  - On-chip PE transposes redistribute [item, (l,k)] rows into matmul-ready
    wt[k, l*128+item]; ACT drains transpose PSUM -> SBUF.
  - Per item: stationary lhsT = wT (strided AP), streamed rhs = xT[128,64];
    PSUM [l, 512] per 8 items; single matmul per psum region (clean groups).
  - Bias gathered transposed (tiny: 1 desc/idx) as B[l, item]; drain =
    DVE tensor_tensor broadcast-add (stride-0 AP) fused with f32->bf16 cast.
"""
import numpy as np
import ml_dtypes
from contextlib import ExitStack

from concourse import bass, bacc, mybir
import concourse.tile as tile
from concourse.bass_utils import run_bass_kernel_spmd
from concourse.library_config import mlp

P = 128          # partitions / OUT_F / IN_F
V = 4096         # nodes
EC = 129         # rows per table element (128 weight rows + bias row)
E = EC * P       # elements per table entry
J = 32           # samples per index
JI = 64          # samples per item (G=2 indices)
K = 128          # contraction size
I_FULL = 8192
N_CORES = 8
CH = 128         # items per chunk
N_ITEMS = 768    # per-core item capacity (6 chunks)

BF16 = mybir.dt.bfloat16
F32 = mybir.dt.float32


def build_program(n_items=N_ITEMS, ji=JI, reps=1, ch=CH):
    nchunk = n_items // ch
    cw = ch * ji                  # output columns per chunk
    nc = bacc.Bacc("TRN2", target_bir_lowering=False, debug=False)
    wtab = nc.dram_tensor("wtab", [V, E], BF16, kind="ExternalInput")
    xt = nc.dram_tensor("xt", [nchunk, P, cw], BF16, kind="ExternalInput")
    widx = nc.dram_tensor(
        "widx", [P, n_items // 16], mybir.dt.int16, kind="ExternalInput"
    )
    # bias-gather indices: same nodes but 0-padded (the transposed bias
    # gather reads garbage rather than skipping for negative indices)
    widx_b = nc.dram_tensor(
        "widx_b", [P, n_items // 16], mybir.dt.int16, kind="ExternalInput"
    )
    # per-chunk count of real (non-negative) weight-gather indices
    wcnt = nc.dram_tensor("wcnt", [1, nchunk], mybir.dt.int32, kind="ExternalInput")
    ident_in = nc.dram_tensor("ident", [P, P], BF16, kind="ExternalInput")
    ydev = nc.dram_tensor("ydev", [nchunk, P, cw], BF16, kind="ExternalOutput")

    with tile.TileContext(nc) as tc:
        with ExitStack() as ctx:
            const = ctx.enter_context(tc.tile_pool(name="const", bufs=1))
            widx_t = const.tile([P, n_items // 16], mybir.dt.int16)
            nc.sync.dma_start(widx_t[:], widx[:])
            widx_bt = const.tile([P, n_items // 16], mybir.dt.int16)
            nc.sync.dma_start(widx_bt[:], widx_b[:])
            wcnt_t = const.tile([1, nchunk], mybir.dt.int32)
            nc.sync.dma_start(wcnt_t[:], wcnt[:])
            cnt_reg = nc.alloc_register(mybir.EngineType.Pool, "gcnt")
            ident = const.tile([P, P], BF16)
            nc.sync.dma_start(ident[:], ident_in[:])

            nc.gpsimd.load_library(mlp)

            wgp = ctx.enter_context(tc.tile_pool(name="wgp", bufs=2))
            wtp = ctx.enter_context(tc.tile_pool(name="wtp", bufs=2))
            bbp = ctx.enter_context(tc.tile_pool(name="bbp", bufs=2))
            xtp = ctx.enter_context(tc.tile_pool(name="xtp", bufs=2))
            outp = ctx.enter_context(tc.tile_pool(name="outp", bufs=2))
            ps_t = ctx.enter_context(tc.tile_pool(name="ps_t", bufs=3, space="PSUM"))
            ps_y = ctx.enter_context(tc.tile_pool(name="ps_y", bufs=2, space="PSUM"))

            def load_and_transpose(c):
                """Gathers + PE transposes for chunk c; returns tiles the
                matmul stage consumes."""
                idx_sl = widx_t[:, c * (ch // 16) : (c + 1) * (ch // 16)]
                idx_b_sl = widx_bt[:, c * (ch // 16) : (c + 1) * (ch // 16)]

                xt_tile = xtp.tile([P, cw], BF16, tag="xt")
                nc.sync.dma_start(xt_tile[:], xt[c])

                # weights: one contiguous 32KB read per item ->
                # wg[item(part), l*128 + k]
                wg = wgp.tile([P, K * P], BF16, tag="wg")
                wg_ap = bass.AP(
                    wg[:].tensor,
                    wg[:].offset,
                    [wg[:].ap[0], [K * P, 1], [1, K * P]],
                )
                nc.gpsimd.reg_load(cnt_reg, wcnt_t[0:1, c : c + 1])
                nc.gpsimd.dma_gather(
                    wg_ap,
                    wtab[:, : K * P],
                    idx_sl,
                    ch,
                    cnt_reg,
                    K * P,
                    elem_step=E,
                )
                # bias row, transposed gather (1 desc/item) -> bb[l, item]
                bb = bbp.tile([P, ch], BF16, tag="bb")
                bb_ap = bass.AP(
                    bb[:].tensor,
                    bb[:].offset,
                    [bb[:].ap[0], [ch, 1], [1, ch]],
                )
                nc.gpsimd.dma_gather(
                    bb_ap,
                    wtab[:, K * P :],
                    idx_b_sl,
                    ch,
                    ch,
                    P,
                    elem_step=E,
                    transpose=True,
                )

                # PE transposes: wg[item, l*128+k] -> wt[k, l*128+item]
                # 8 transposes per one-bank PSUM tile, one big drain each
                wt = wtp.tile([P, K * P], BF16, tag="wt")
                for g in range(K // 8):
                    pst = ps_t.tile([P, 8 * P], BF16, tag="pt")
                    for u in range(8):
                        l = 8 * g + u
                        nc.tensor.transpose(
                            pst[:, u * P : (u + 1) * P],
                            wg[:, l * P : (l + 1) * P],
                            ident[:],
                        )
                    # split PSUM->SBUF drains ~6:10 DVE:ACT (DVE also carries
                    # the bias-add output drains)
                    if g % 8 in (0, 3, 6):
                        nc.vector.tensor_copy(
                            wt[:, 8 * g * P : (8 * g + 8) * P], pst[:]
                        )
                    else:
                        nc.scalar.activation(
                            wt[:, 8 * g * P : (8 * g + 8) * P],
                            pst[:],
                            mybir.ActivationFunctionType.Copy,
                        )
                return xt_tile, wt, bb

            def matmul_and_out(c, staged):
                xt_tile, wt, bb = staged
                yout = outp.tile([P, cw], BF16, tag="yo")
                gi = 1024 // ji                 # items per (2-bank) psum group
                for g in range(cw // 1024):
                    ypsum = ps_y.tile([P, 1024], F32, tag="yp")
                    for u in range(gi):
                        it = g * gi + u
                        lhsT = bass.AP(
                            wt[:].tensor,
                            wt[:].offset + it,
                            [wt[:].ap[0], [ch, P]],
                        )
                        nc.tensor.matmul(
                            ypsum[:, u * ji : (u + 1) * ji],
                            lhsT,
                            xt_tile[:, it * ji : (it + 1) * ji],
                            start=True,
                            stop=True,
                        )
                    bias_bc = bass.AP(
                        bb[:].tensor,
                        bb[:].offset + g * gi,
                        [bb[:].ap[0], [1, gi], [0, ji]],
                    )
                    nc.vector.tensor_tensor(
                        yout[:, g * 1024 : (g + 1) * 1024],
                        ypsum[:],
                        bias_bc,
                        op=mybir.AluOpType.add,
                    )
                nc.sync.dma_start(ydev[c], yout[:])

            # software pipeline: issue chunk c+1's gather+transposes before
            # chunk c's matmuls so the PE never head-of-line blocks on the
            # transpose drains
            staged = None
            total = reps * nchunk
            for n in range(total + 1):
                if n < total:
                    nxt = load_and_transpose(n % nchunk)
                if staged is not None:
                    matmul_and_out((n - 1) % nchunk, staged)
                staged = nxt if n < total else None
    nc.compile()
    return nc


def make_tabs(params):
    """params [V, 128, 129] f32 -> wtab [V, 129*128] bf16: l-major [l, k]
    weight block then the bias vector as a 129th row."""
    w = params[:, :, :K].reshape(V, P * K)
    b = params[:, :, K]
    return np.ascontiguousarray(np.concatenate([w, b], axis=1)).astype(
        ml_dtypes.bfloat16
    )


def make_consts():
    return np.eye(P, dtype=ml_dtypes.bfloat16)


def wrap_idxs(ids):
    n = len(ids)
    w = np.asarray(ids, np.int16).reshape(n // 16, 16).T
    return np.tile(w, (8, 1))


def route_and_group(inds, n_cores=N_CORES, n_items=N_ITEMS):
    """Sort indices by node, split into per-core runs at node boundaries,
    group <=2 same-node indices per item.

    Returns per-core: item_nodes [n_items], and (item, slot) -> original
    index position maps (orig_pos, item_slot arrays).
    """
    inds = np.asarray(inds).astype(np.int64)
    I = len(inds)
    order = np.argsort(inds, kind="stable")
    s = inds[order]
    bounds = [0]
    for c in range(1, n_cores):
        t = c * I // n_cores
        while t < I and s[t] == s[t - 1]:
            t += 1
        bounds.append(t)
    bounds.append(I)

    cores = []
    for c in range(n_cores):
        seg_pos = order[bounds[c] : bounds[c + 1]]   # original positions
        seg_node = s[bounds[c] : bounds[c + 1]]
        item_nodes = []
        pair_pos = []          # list of (orig_pos, item, slot)
        i = 0
        n = len(seg_node)
        while i < n:
            node = seg_node[i]
            item = len(item_nodes)
            item_nodes.append(node)
            pair_pos.append((seg_pos[i], item, 0))
            if i + 1 < n and seg_node[i + 1] == node:
                pair_pos.append((seg_pos[i + 1], item, 1))
                i += 2
            else:
                i += 1
        if len(item_nodes) > n_items:
            raise ValueError(
                f"core {c}: {len(item_nodes)} items exceed capacity {n_items}"
            )
        npad = n_items - len(item_nodes)
        # -1 pads: the (non-transposed) weight gather skips trailing
        # negative indices, saving their DMA traffic entirely
        item_nodes = np.asarray(item_nodes + [-1] * npad, np.int64)
        # keep at least one real index per gather call (avoid the
        # all-negative edge case on fully-padded chunks)
        for cc in range(0, n_items, 128):
            if (item_nodes[cc : cc + 128] < 0).all():
                item_nodes[cc] = 0
        pp = np.asarray(pair_pos, np.int64)
        cores.append((item_nodes, pp))
    return cores


def host_pre_core(x_full_bf16, item_nodes, pair_pos, n_items=N_ITEMS, ji=JI,
                  ch=CH):
    """Build xt [nchunk, 128, ch*ji], widx (-1 pads), widx_b (0 pads) and
    per-chunk real counts for one core."""
    nchunk = n_items // ch
    # xs[item, slot] = x rows; pad slots zero
    xs = np.zeros((n_items * 2, J, K), ml_dtypes.bfloat16)
    slot_idx = pair_pos[:, 1] * 2 + pair_pos[:, 2]
    xs[slot_idx] = x_full_bf16[pair_pos[:, 0]]
    # -> [nchunk, ch, ji(=2*J), K] -> [nchunk, K, ch*ji]
    xs = xs.reshape(nchunk, ch, ji, K).transpose(0, 3, 1, 2).reshape(
        nchunk, K, ch * ji
    )
    xt = np.ascontiguousarray(xs)
    widx = np.hstack(
        [wrap_idxs(item_nodes[c * ch : (c + 1) * ch]) for c in range(nchunk)]
    ).astype(np.int16)
    nodes_b = np.where(item_nodes < 0, 0, item_nodes)
    widx_b = np.hstack(
        [wrap_idxs(nodes_b[c * ch : (c + 1) * ch]) for c in range(nchunk)]
    ).astype(np.int16)
    wcnt = np.asarray(
        [
            int((item_nodes[c * ch : (c + 1) * ch] >= 0).sum())
            for c in range(nchunk)
        ],
        np.int32,
    ).reshape(1, nchunk)
    return xt, widx, widx_b, wcnt


def host_post(ydevs, cores, n_items=N_ITEMS, ji=JI, ch=CH):
    """ydev[c][l, item*ji + slot*J + j] -> out[orig, j, l]."""
    out = np.empty((I_FULL, J, P), np.float32)
    nchunk = n_items // ch
    for core, (item_nodes, pair_pos) in enumerate(cores):
        y = np.asarray(ydevs[core]).astype(np.float32)
        # [nchunk, P, ch*ji] -> [n_items*2, J, P]
        y = y.reshape(nchunk, P, ch, ji).transpose(0, 2, 3, 1).reshape(
            n_items * 2, J, P
        )
        slot_idx = pair_pos[:, 1] * 2 + pair_pos[:, 2]
        out[pair_pos[:, 0]] = y[slot_idx]
    return out


_NC_CACHE = {}


def get_program(n_items=N_ITEMS, ji=JI, reps=1, ch=CH):
    key = (n_items, ji, reps, ch)
    if key not in _NC_CACHE:
        _NC_CACHE[key] = build_program(n_items, ji, reps, ch)
    return _NC_CACHE[key]


def make_in_maps(x_in, inds_ne, params, n_items=N_ITEMS):
    wtab = make_tabs(np.asarray(params, dtype=np.float32))
    ident = make_consts()
    x_bf16 = np.asarray(x_in, np.float32).astype(ml_dtypes.bfloat16)
    cores = route_and_group(inds_ne, N_CORES, n_items)
    in_maps = []
    for c in range(N_CORES):
        item_nodes, pair_pos = cores[c]
        xtc, widx, widx_b, wcnt = host_pre_core(x_bf16, item_nodes, pair_pos,
                                                n_items)
        in_maps.append(
            {
                "wtab": wtab,
                "xt": xtc,
                "widx": widx,
                "widx_b": widx_b,
                "wcnt": wcnt,
                "ident": ident,
            }
        )
    return in_maps, cores


def kernel(x_in, inds_ne, params):
    x_in = np.asarray(x_in, dtype=np.float32)
    inds_ne = np.asarray(inds_ne)
    params = np.asarray(params, dtype=np.float32)

    n_items = N_ITEMS
    while True:
        try:
            in_maps, cores = make_in_maps(x_in, inds_ne, params, n_items)
            break
        except ValueError:
            n_items += CH
    nc = get_program(n_items)
    res = run_bass_kernel_spmd(nc, in_maps, core_ids=list(range(N_CORES)))
    return host_post([res.results[c]["ydev"] for c in range(N_CORES)], cores,
                     n_items)


# revision 6
# speedup vs baseline: 2.1812x; 2.1812x over previous
"""Trainium2 Bass kernel for nn_MultiDense (moe_routing) — v8.

v7 + two-pass items to eliminate x/output padding waste:
  - "doubles" pass: items = 2 same-node indices (64 samples, ji=64)
  - "singles" pass: items = 1 index (32 samples, ji=32)
Items are dealt in sorted order across cores in equal blocks (384 doubles +
384 singles capacity per core, 3+3 chunks of 128); a node's items may land
on different cores (each item gathers its own weights anyway). x/out DMA
bytes become exact instead of 2x-padded. Weight gathers still skip trailing
-1 pads via runtime counts.
"""
import numpy as np
import ml_dtypes
from contextlib import ExitStack

from concourse import bass, bacc, mybir
import concourse.tile as tile
from concourse.bass_utils import run_bass_kernel_spmd
from concourse.library_config import mlp

P = 128          # partitions / OUT_F / IN_F
V = 4096         # nodes
EC = 129         # rows per table element (128 weight rows + bias row)
E = EC * P       # elements per table entry
J = 32           # samples per index
K = 128          # contraction size
I_FULL = 8192
N_CORES = 8
CH = 128         # items per chunk
ND = 384         # per-core double-item capacity (3 chunks)
NS = 384         # per-core single-item capacity (3 chunks)

BF16 = mybir.dt.bfloat16
F32 = mybir.dt.float32


def build_program(nd=ND, ns=NS, reps=1, ch=CH):
    ncd = nd // ch
    ncs = ns // ch
    nchunks = ncd + ncs
    nc = bacc.Bacc("TRN2", target_bir_lowering=False, debug=False)
    wtab = nc.dram_tensor("wtab", [V, E], BF16, kind="ExternalInput")
    xt_d = nc.dram_tensor("xt_d", [ncd, P, ch * 64], BF16, kind="ExternalInput")
    xt_s = nc.dram_tensor("xt_s", [ncs, P, ch * 32], BF16, kind="ExternalInput")
    widx_d = nc.dram_tensor("widx_d", [P, nd // 16], mybir.dt.int16,
                            kind="ExternalInput")
    widx_db = nc.dram_tensor("widx_db", [P, nd // 16], mybir.dt.int16,
                             kind="ExternalInput")
    widx_s = nc.dram_tensor("widx_s", [P, ns // 16], mybir.dt.int16,
                            kind="ExternalInput")
    widx_sb = nc.dram_tensor("widx_sb", [P, ns // 16], mybir.dt.int16,
                             kind="ExternalInput")
    wcnt = nc.dram_tensor("wcnt", [1, nchunks], mybir.dt.int32,
                          kind="ExternalInput")
    ident_in = nc.dram_tensor("ident", [P, P], BF16, kind="ExternalInput")
    ydev_d = nc.dram_tensor("ydev_d", [ncd, P, ch * 64], BF16,
                            kind="ExternalOutput")
    ydev_s = nc.dram_tensor("ydev_s", [ncs, P, ch * 32], BF16,
                            kind="ExternalOutput")

    # chunk schedule: (ji, local chunk, xt dram, ydev dram, widx pair)
    with tile.TileContext(nc) as tc:
        with ExitStack() as ctx:
            const = ctx.enter_context(tc.tile_pool(name="const", bufs=1))
            widx_dt = const.tile([P, nd // 16], mybir.dt.int16)
            nc.sync.dma_start(widx_dt[:], widx_d[:])
            widx_dbt = const.tile([P, nd // 16], mybir.dt.int16)
            nc.sync.dma_start(widx_dbt[:], widx_db[:])
            widx_st = const.tile([P, ns // 16], mybir.dt.int16)
            nc.sync.dma_start(widx_st[:], widx_s[:])
            widx_sbt = const.tile([P, ns // 16], mybir.dt.int16)
            nc.sync.dma_start(widx_sbt[:], widx_sb[:])
            wcnt_t = const.tile([1, nchunks], mybir.dt.int32)
            nc.sync.dma_start(wcnt_t[:], wcnt[:])
            ident = const.tile([P, P], BF16)
            nc.sync.dma_start(ident[:], ident_in[:])
            cnt_reg = nc.alloc_register(mybir.EngineType.Pool, "gcnt")

            nc.gpsimd.load_library(mlp)

            wgp = ctx.enter_context(tc.tile_pool(name="wgp", bufs=2))
            wtp = ctx.enter_context(tc.tile_pool(name="wtp", bufs=2))
            bbp = ctx.enter_context(tc.tile_pool(name="bbp", bufs=2))
            xtp = ctx.enter_context(tc.tile_pool(name="xtp", bufs=2))
            outp = ctx.enter_context(tc.tile_pool(name="outp", bufs=2))
            ps_t = ctx.enter_context(tc.tile_pool(name="ps_t", bufs=3, space="PSUM"))
            ps_y = ctx.enter_context(tc.tile_pool(name="ps_y", bufs=2, space="PSUM"))

            chunks = [
                ("d", c, xt_d, ydev_d, widx_dt, widx_dbt, 64, c)
                for c in range(ncd)
            ] + [
                ("s", c, xt_s, ydev_s, widx_st, widx_sbt, 32, ncd + c)
                for c in range(ncs)
            ]

            def load_and_transpose(chunk):
                _, c, xtd, _, wix, wixb, ji, cnt_i = chunk
                cw = ch * ji
                idx_sl = wix[:, c * (ch // 16) : (c + 1) * (ch // 16)]
                idx_b_sl = wixb[:, c * (ch // 16) : (c + 1) * (ch // 16)]

                xt_tile = xtp.tile([P, ch * 64], BF16, tag="xt")
                nc.sync.dma_start(xt_tile[:, :cw], xtd[c])

                # weights: one contiguous 32KB read per item ->
                # wg[item(part), l*128 + k]
                wg = wgp.tile([P, K * P], BF16, tag="wg")
                wg_ap = bass.AP(
                    wg[:].tensor,
                    wg[:].offset,
                    [wg[:].ap[0], [K * P, 1], [1, K * P]],
                )
                nc.gpsimd.reg_load(cnt_reg, wcnt_t[0:1, cnt_i : cnt_i + 1])
                nc.gpsimd.dma_gather(
                    wg_ap,
                    wtab[:, : K * P],
                    idx_sl,
                    ch,
                    cnt_reg,
                    K * P,
                    elem_step=E,
                )
                # bias row, transposed gather (1 desc/item) -> bb[l, item]
                bb = bbp.tile([P, ch], BF16, tag="bb")
                bb_ap = bass.AP(
                    bb[:].tensor,
                    bb[:].offset,
                    [bb[:].ap[0], [ch, 1], [1, ch]],
                )
                nc.gpsimd.dma_gather(
                    bb_ap,
                    wtab[:, K * P :],
                    idx_b_sl,
                    ch,
                    ch,
                    P,
                    elem_step=E,
                    transpose=True,
                )

                # PE transposes: wg[item, l*128+k] -> wt[k, l*128+item]
                # 8 transposes per one-bank PSUM tile, one big drain each
                wt = wtp.tile([P, K * P], BF16, tag="wt")
                for g in range(K // 8):
                    pst = ps_t.tile([P, 8 * P], BF16, tag="pt")
                    for u in range(8):
                        l = 8 * g + u
                        nc.tensor.transpose(
                            pst[:, u * P : (u + 1) * P],
                            wg[:, l * P : (l + 1) * P],
                            ident[:],
                        )
                    # split PSUM->SBUF drains ~6:10 DVE:ACT (DVE also carries
                    # the bias-add output drains)
                    if g % 8 in (0, 3, 6):
                        nc.vector.tensor_copy(
                            wt[:, 8 * g * P : (8 * g + 8) * P], pst[:]
                        )
                    else:
                        nc.scalar.activation(
                            wt[:, 8 * g * P : (8 * g + 8) * P],
                            pst[:],
                            mybir.ActivationFunctionType.Copy,
                        )
                return xt_tile, wt, bb

            def matmul_and_out(chunk, staged):
                _, c, _, ydevd, _, _, ji, _ = chunk
                cw = ch * ji
                xt_tile, wt, bb = staged
                yout = outp.tile([P, ch * 64], BF16, tag="yo")
                gi = 1024 // ji                 # items per (2-bank) psum group
                for g in range(cw // 1024):
                    ypsum = ps_y.tile([P, 1024], F32, tag="yp")
                    for u in range(gi):
                        it = g * gi + u
                        lhsT = bass.AP(
                            wt[:].tensor,
                            wt[:].offset + it,
                            [wt[:].ap[0], [ch, P]],
                        )
                        nc.tensor.matmul(
                            ypsum[:, u * ji : (u + 1) * ji],
                            lhsT,
                            xt_tile[:, it * ji : (it + 1) * ji],
                            start=True,
                            stop=True,
                        )
                    bias_bc = bass.AP(
                        bb[:].tensor,
                        bb[:].offset + g * gi,
                        [bb[:].ap[0], [1, gi], [0, ji]],
                    )
                    nc.vector.tensor_tensor(
                        yout[:, g * 1024 : (g + 1) * 1024],
                        ypsum[:],
                        bias_bc,
                        op=mybir.AluOpType.add,
                    )
                nc.sync.dma_start(ydevd[c], yout[:, :cw])

            # software pipeline: issue chunk n+1's gather+transposes before
            # chunk n's matmuls so the PE never head-of-line blocks on the
            # transpose drains
            staged = None
            total = reps * len(chunks)
            prev = None
            for n in range(total + 1):
                if n < total:
                    cur = chunks[n % len(chunks)]
                    nxt = load_and_transpose(cur)
                if staged is not None:
                    matmul_and_out(prev, staged)
                staged = nxt if n < total else None
                prev = cur if n < total else None
    nc.compile()
    return nc


def make_tabs(params):
    """params [V, 128, 129] f32 -> wtab [V, 129*128] bf16: l-major [l, k]
    weight block then the bias vector as a 129th row."""
    w = params[:, :, :K].reshape(V, P * K)
    b = params[:, :, K]
    return np.ascontiguousarray(np.concatenate([w, b], axis=1)).astype(
        ml_dtypes.bfloat16
    )


def make_consts():
    return np.eye(P, dtype=ml_dtypes.bfloat16)


def wrap_idxs(ids):
    n = len(ids)
    w = np.asarray(ids, np.int16).reshape(n // 16, 16).T
    return np.tile(w, (8, 1))


def route_and_group(inds, nd=ND, ns=NS, n_cores=N_CORES):
    """Global node-sort; emit double items (2 same-node indices) and single
    items; deal consecutive equal blocks to cores.

    Returns per-core (d_nodes [nd], s_nodes [ns], d_pairs [m,3], s_pairs
    [m,2]) where pairs map (orig position -> item, slot)."""
    inds = np.asarray(inds).astype(np.int64)
    I = len(inds)
    order = np.argsort(inds, kind="stable")
    s = inds[order]

    d_nodes_g, d_pairs_g = [], []   # (pos, item, slot)
    s_nodes_g, s_pairs_g = [], []   # (pos, item)
    i = 0
    while i < I:
        node = s[i]
        j = i
        while j < I and s[j] == node:
            j += 1
        k = j - i
        for t in range(k // 2):
            item = len(d_nodes_g)
            d_nodes_g.append(node)
            d_pairs_g.append((order[i + 2 * t], item, 0))
            d_pairs_g.append((order[i + 2 * t + 1], item, 1))
        if k % 2:
            item = len(s_nodes_g)
            s_nodes_g.append(node)
            s_pairs_g.append((order[i + k - 1], item))
        i = j

    if len(d_nodes_g) > nd * n_cores:
        raise ValueError(f"{len(d_nodes_g)} doubles exceed {nd * n_cores}")
    if len(s_nodes_g) > ns * n_cores:
        raise ValueError(f"{len(s_nodes_g)} singles exceed {ns * n_cores}")

    d_nodes_g = np.asarray(d_nodes_g, np.int64)
    s_nodes_g = np.asarray(s_nodes_g, np.int64)
    d_pairs_g = np.asarray(d_pairs_g, np.int64).reshape(-1, 3)
    s_pairs_g = np.asarray(s_pairs_g, np.int64).reshape(-1, 2)

    def pad_block(nodes_g, cap, c):
        # round-robin deal: core c takes items c, c+8, c+16, ... (balanced)
        blk = nodes_g[c::n_cores][:cap]
        out = np.full(cap, -1, np.int64)
        out[: len(blk)] = blk
        # keep >=1 real index per 128-chunk gather call
        for cc in range(0, cap, 128):
            if (out[cc : cc + 128] < 0).all():
                out[cc] = 0
        return out

    cores = []
    for c in range(n_cores):
        d_nodes = pad_block(d_nodes_g, nd, c)
        s_nodes = pad_block(s_nodes_g, ns, c)
        dm = d_pairs_g[:, 1] % n_cores == c
        sm = s_pairs_g[:, 1] % n_cores == c
        dp = d_pairs_g[dm].copy()
        dp[:, 1] //= n_cores
        sp = s_pairs_g[sm].copy()
        sp[:, 1] //= n_cores
        cores.append((d_nodes, s_nodes, dp, sp))
    return cores


def host_pre_core(x_bf16, core, nd=ND, ns=NS, ch=CH):
    d_nodes, s_nodes, dp, sp = core
    ncd, ncs = nd // ch, ns // ch
    xs_d = np.zeros((nd * 2, J, K), ml_dtypes.bfloat16)
    xs_d[dp[:, 1] * 2 + dp[:, 2]] = x_bf16[dp[:, 0]]
    xt_d = np.ascontiguousarray(
        xs_d.reshape(ncd, ch, 64, K).transpose(0, 3, 1, 2).reshape(ncd, K, ch * 64)
    )
    xs_s = np.zeros((ns, J, K), ml_dtypes.bfloat16)
    xs_s[sp[:, 1]] = x_bf16[sp[:, 0]]
    xt_s = np.ascontiguousarray(
        xs_s.reshape(ncs, ch, 32, K).transpose(0, 3, 1, 2).reshape(ncs, K, ch * 32)
    )

    def wr(nodes):
        return np.hstack(
            [wrap_idxs(nodes[c * ch : (c + 1) * ch]) for c in range(len(nodes) // ch)]
        ).astype(np.int16)

    widx_d = wr(d_nodes)
    widx_db = wr(np.where(d_nodes < 0, 0, d_nodes))
    widx_s = wr(s_nodes)
    widx_sb = wr(np.where(s_nodes < 0, 0, s_nodes))
    cnts = [int((d_nodes[c * ch : (c + 1) * ch] >= 0).sum()) for c in range(ncd)]
    cnts += [int((s_nodes[c * ch : (c + 1) * ch] >= 0).sum()) for c in range(ncs)]
    wcnt = np.asarray(cnts, np.int32).reshape(1, ncd + ncs)
    return {
        "xt_d": xt_d,
        "xt_s": xt_s,
        "widx_d": widx_d,
        "widx_db": widx_db,
        "widx_s": widx_s,
        "widx_sb": widx_sb,
        "wcnt": wcnt,
    }


def host_post(ydevs_d, ydevs_s, cores, nd=ND, ns=NS, ch=CH):
    out = np.empty((I_FULL, J, P), np.float32)
    ncd, ncs = nd // ch, ns // ch
    for core in range(N_CORES):
        d_nodes, s_nodes, dp, sp = cores[core]
        yd = np.asarray(ydevs_d[core]).astype(np.float32)
        yd = yd.reshape(ncd, P, ch, 64).transpose(0, 2, 3, 1).reshape(nd * 2, J, P)
        out[dp[:, 0]] = yd[dp[:, 1] * 2 + dp[:, 2]]
        ys = np.asarray(ydevs_s[core]).astype(np.float32)
        ys = ys.reshape(ncs, P, ch, 32).transpose(0, 2, 3, 1).reshape(ns, J, P)
        out[sp[:, 0]] = ys[sp[:, 1]]
    return out


_NC_CACHE = {}


def get_program(nd=ND, ns=NS, reps=1, ch=CH):
    key = (nd, ns, reps, ch)
    if key not in _NC_CACHE:
        _NC_CACHE[key] = build_program(nd, ns, reps, ch)
    return _NC_CACHE[key]


def make_in_maps(x_in, inds_ne, params, nd=ND, ns=NS):
    wtab = make_tabs(np.asarray(params, dtype=np.float32))
    ident = make_consts()
    x_bf16 = np.asarray(x_in, np.float32).astype(ml_dtypes.bfloat16)
    cores = route_and_group(inds_ne, nd, ns)
    in_maps = []
    for c in range(N_CORES):
        m = host_pre_core(x_bf16, cores[c], nd, ns)
        m["wtab"] = wtab
        m["ident"] = ident
        in_maps.append(m)
    return in_maps, cores


def kernel(x_in, inds_ne, params):
    x_in = np.asarray(x_in, dtype=np.float32)
    inds_ne = np.asarray(inds_ne)
    params = np.asarray(params, dtype=np.float32)

    nd, ns = ND, NS
    while True:
        try:
            in_maps, cores = make_in_maps(x_in, inds_ne, params, nd, ns)
            break
        except ValueError as e:
            if "doubles" in str(e):
                nd += CH
            else:
                ns += CH
    nc = get_program(nd, ns)
    res = run_bass_kernel_spmd(nc, in_maps, core_ids=list(range(N_CORES)))
    return host_post(
        [res.results[c]["ydev_d"] for c in range(N_CORES)],
        [res.results[c]["ydev_s"] for c in range(N_CORES)],
        cores,
        nd,
        ns,
    )
